# revision 13
# baseline (speedup 1.0000x reference)
"""Trainium2 Bass kernel for nn_LogicalGNNLayer (GNN message passing + MLP).

Computation (reference):
    h = term_emb[heads]; t = term_emb[tails]           # gather  [E,B,D]
    agg = segsum(s*(h+pred), tails) + segsum(s*(t+inv), heads)   # [T,B,D]
    agg += EPS*term_emb
    out = relu(agg @ W1 + b1) @ W2 + b2                # [T,B,D]

Strategy (v1 rewrite):
  - Batch B sharded across 8 cores (Bc=512); term/edge structure and weights
    replicated.  heads/tails/signs are read on the host and the message
    structure is baked into the program.
  - Fused on-chip layout: 128 partitions hold d-within-tile, the free axis
    holds (k, dt, b) for terms (so each term/acc is one contiguous
    [128, 1024] span) and (dt, k, b) for the output.
  - The aggregation acc[k] = (EPS + selfs)*term[k] + sum c*term[src] + sum emb
    is elementwise work balanced across THREE engines (HW-calibrated costs):
      * DVE:  tensor ops at 2x fp16 mode (~716ns per [128,1024] add)
      * Pool: tensor ops (~2.08us/add; slow but otherwise idle)
      * PE:   scaled-identity matmuls accumulating in PSUM (~174ns/512 cols),
              merged to SBUF by one Act-engine copy per group
  - emb tensors for PE/Pool groups are cast to fp8e3 (e3m4) on the host,
    halving their DMA traffic at no engine cost; DVE groups keep fp16 (fp8
    operands drop DVE to 1x mode).  Per-group dtype is a balance knob.
  - MLP: fp16 matmuls with fp32 PSUM accumulation; ReLU+b1 / +b2 epilogues
    on the Act engine via bias APs (nonzero biases handled), with an
    adjustable DVE share.
  - Output stored fp16 (cast to fp32 on the host).
"""

import numpy as np

import concourse.bass as bass
import concourse.tile as tile
from concourse import bacc, mybir
from concourse.bass_utils import run_bass_kernel_spmd

T, B, D, H, E = 16, 4096, 256, 512, 32
EPS = 0.1
N_CORES = 8
BC = B // N_CORES            # 512 batch per core
DT = D // 128                # 2 d-tiles
HT = H // 128                # 4 h-tiles
KC = DT * BC                 # 1024 columns per term (dt-fused)
NCOL = T * KC                # 16384 free-axis columns
ECHUNK = 8                   # messages per emb DMA/tile
F16 = mybir.dt.float16
F32 = mybir.dt.float32
F8 = mybir.dt.float8e3       # e3m4: 4 mantissa bits
AF = mybir.ActivationFunctionType
OP = mybir.AluOpType
ET = mybir.EngineType

# HW-calibrated per-op costs in us ([128,1024]-equivalent; microbenched on
# the axon-tunneled trn2):
C_DVE_ADD = 0.716
C_DVE_ADD8 = 1.295
C_DVE_STT = 1.35
C_DVE_INIT = 0.39
C_DVE_EPI = 1.43
C_POOL_ADD = 2.08
C_POOL_STT = 1.55
C_PE_512 = 0.174
C_ACT_TILE = 1.04
C_MLP = 44.5
C_ACT_FIXED = 48 * 1.037     # 32 relu + 16 out tiles, all on Act
US_PER_MIB = 2.697
DMA_FIXED_MIB = 4.0 + 4.0 + 0.625 + 0.07   # term + out + weights + ident
C_SP_DMA = 0.565

_KERNEL_CACHE = {}


def _messages(heads, tails, signs):
    """Directed message list (dst, src, sign, which_emb, e)."""
    msgs = []
    for e in range(E):
        h, t, s = int(heads[e]), int(tails[e]), float(signs[e])
        assert 0 <= h < T and 0 <= t < T
        msgs.append((t, h, s, 0, e))   # msg_to_tail: acc[t] += s*(term[h]+pred[e])
        msgs.append((h, t, s, 1, e))   # msg_to_head: acc[h] += s*(term[t]+inv[e])
    return msgs


def _plan(msgs_key):
    """Group messages by destination, assign each group to an engine, lay
    out the emb DRAM ordering, and pick the epilogue split — all from
    HW-calibrated cost estimates."""
    msgs = list(msgs_key)
    groups = []
    for k in range(T):
        mlist = [(i, m) for i, m in enumerate(msgs) if m[0] == k]
        self_coeff = EPS + sum(m[2] for _, m in mlist if m[1] == k)
        pair_coeffs = {}
        for _, (_d, src, s, _w, _e) in mlist:
            if src != k:
                pair_coeffs[src] = pair_coeffs.get(src, 0.0) + s
        pairs = [(src, c) for src, c in sorted(pair_coeffs.items()) if c != 0.0]
        emb = [i for i, _m in mlist]   # every message carries its emb term
        groups.append(dict(k=k, self_coeff=self_coeff, pairs=pairs, emb=emb))

    def dve_cost(g):
        c = sum(C_DVE_ADD if abs(co) == 1.0 else C_DVE_STT
                for _s, co in g["pairs"])
        return c + len(g["emb"]) * C_DVE_ADD

    def pool_cost(g):
        # Pool lacks TensorScalarPtr: |coeff|==c emitted as c adds/subs
        c = sum(abs(co) * C_POOL_ADD if float(co).is_integer() else C_DVE_STT
                for _s, co in g["pairs"])
        return c + len(g["emb"]) * C_POOL_ADD

    def pe_cost(g):
        return (2 + 2 * len(g["pairs"]) + 2 * len(g["emb"])) * C_PE_512

    est = {"pe": C_MLP, "dve": T * C_DVE_INIT, "pool": 0.0,
           "act": C_ACT_FIXED, "dma": 0.0, "sp": 40 * C_SP_DMA}
    emb_mib = 0.0
    assign = {}
    order = sorted(range(T), key=lambda k: -dve_cost(groups[k]))
    for k in order:
        g = groups[k]
        nmsg = len(g["emb"])
        dma16 = (DMA_FIXED_MIB + emb_mib + 0.25 * nmsg) * US_PER_MIB
        dma8 = (DMA_FIXED_MIB + emb_mib + 0.125 * nmsg) * US_PER_MIB
        cand = {
            "dve": max(est["pe"], est["dve"] + dve_cost(g), est["pool"],
                       est["act"], dma16),
            # Pool reading fp8 crashes the device (NRT_EXEC_UNIT_UNRECOVERABLE)
            # so pool groups keep fp16 emb.
            "pool": max(est["pe"], est["dve"], est["pool"] + pool_cost(g),
                        est["act"], dma16),
            "pe": max(est["pe"] + pe_cost(g), est["dve"] - C_DVE_INIT,
                      est["pool"], est["act"] + C_ACT_TILE, dma8),
        }
        eng = min(cand, key=lambda e: cand[e])
        assign[k] = eng
        if eng == "dve":
            est["dve"] += dve_cost(g)
            emb_mib += 0.25 * nmsg
        elif eng == "pool":
            est["pool"] += pool_cost(g)
            emb_mib += 0.25 * nmsg
        else:
            est["pe"] += pe_cost(g)
            est["act"] += C_ACT_TILE
            est["dve"] -= C_DVE_INIT
            emb_mib += 0.125 * nmsg
    est["dma"] = (DMA_FIXED_MIB + emb_mib) * US_PER_MIB

    # epilogue split: move tiles Act -> DVE while the makespan improves
    epi_dve = 0
    while epi_dve < 24:
        cur = max(est.values())
        nxt_act, nxt_dve = est["act"] - 1.037, est["dve"] + C_DVE_EPI
        if max(est["pe"], est["pool"], est["dma"], est["sp"],
               nxt_act, nxt_dve) < cur:
            est["act"], est["dve"] = nxt_act, nxt_dve
            epi_dve += 1
        else:
            break

    # flip DVE-group emb to fp8 while DMA dominates (NOT pool: fp8 on the
    # Pool engine crashes the device)
    use8 = {k: assign[k] == "pe" for k in range(T)}
    for k in order:
        if assign[k] != "dve":
            continue
        nmsg = len(groups[k]["emb"])
        if nmsg and est["dma"] == max(est.values()):
            nxt_dve = est["dve"] + (C_DVE_ADD8 - C_DVE_ADD) * nmsg
            nxt_dma = est["dma"] - 0.125 * nmsg * US_PER_MIB
            if max(nxt_dve, nxt_dma) < max(est["dve"], est["dma"]):
                est["dve"], est["dma"] = nxt_dve, nxt_dma
                use8[k] = True

    # emb DRAM layouts: grouped by k in processing order
    f16_order, f8_order = [], []
    for k in range(T):
        (f8_order if use8[k] else f16_order).extend(groups[k]["emb"])

    scales = []
    for k in range(T):
        if assign[k] != "pe":
            continue
        g = groups[k]
        for c in [g["self_coeff"]] + [c for _s, c in g["pairs"]]:
            if c not in scales:
                scales.append(c)

    return dict(groups=groups, assign=assign, use8=use8,
                f16_order=f16_order, f8_order=f8_order, scales=scales,
                epi_dve=epi_dve, est=est, msgs=msgs)


def _build(msgs_key, repeats=1, loop=0):
    key = (msgs_key, repeats, loop)
    if key in _KERNEL_CACHE:
        return _KERNEL_CACHE[key]
    plan = _plan(msgs_key)
    groups, assign, use8 = plan["groups"], plan["assign"], plan["use8"]
    f16_pos = {m: j for j, m in enumerate(plan["f16_order"])}
    f8_pos = {m: j for j, m in enumerate(plan["f8_order"])}
    scales = plan["scales"]
    n16, n8 = len(plan["f16_order"]), len(plan["f8_order"])

    nc = bacc.Bacc("TRN2", target_bir_lowering=False, debug=False,
                   num_devices=N_CORES)
    termT = nc.declare_dram_parameter("termT", [128, NCOL], F16, isOutput=False)
    emb16 = nc.declare_dram_parameter("emb16", [128, max(n16, 1) * KC], F16,
                                      isOutput=False)
    emb8 = nc.declare_dram_parameter("emb8", [128, max(n8, 1) * KC], F8,
                                     isOutput=False)
    w1d = nc.declare_dram_parameter("w1t", [128, DT * H], F16, isOutput=False)
    w2d = nc.declare_dram_parameter("w2t", [128, HT * D], F16, isOutput=False)
    b1d = nc.declare_dram_parameter("b1t", [128, HT], F32, isOutput=False)
    b2d = nc.declare_dram_parameter("b2t", [128, DT], F32, isOutput=False)
    idd = nc.declare_dram_parameter("identT", [128, max(len(scales), 1) * 128],
                                    F16, isOutput=False)
    id8d = nc.declare_dram_parameter("ident8", [128, 128], F8, isOutput=False)
    outT = nc.declare_dram_parameter("outT", [128, NCOL], F16, isOutput=True)

    with nc.allow_low_precision(reason="fp16/fp8 on-chip"), \
            tile.TileContext(nc) as tc, \
            tc.tile_pool(name="const", bufs=1) as cpool, \
            tc.tile_pool(name="term", bufs=2) as tpool, \
            tc.tile_pool(name="acc", bufs=1) as apool, \
            tc.tile_pool(name="e16p", bufs=3) as e16pool, \
            tc.tile_pool(name="e8p", bufs=2) as e8pool, \
            tc.tile_pool(name="hid", bufs=8) as hpool, \
            tc.tile_pool(name="out", bufs=4) as opool, \
            tc.tile_pool(name="ps1", bufs=2, space="PSUM") as ps1pool, \
            tc.tile_pool(name="ps2", bufs=1, space="PSUM") as ps2pool, \
            tc.tile_pool(name="psa", bufs=1, space="PSUM") as psapool:

        # ---- persistent loads -------------------------------------------
        w1s = cpool.tile([128, DT * H], F16, tag="w1")
        nc.sync.dma_start(w1s[:], w1d[:, :])
        w2s = cpool.tile([128, HT * D], F16, tag="w2")
        nc.sync.dma_start(w2s[:], w2d[:, :])
        b1s = cpool.tile([128, HT], F32, tag="b1")
        nc.sync.dma_start(b1s[:], b1d[:, :])
        b2s = cpool.tile([128, DT], F32, tag="b2")
        nc.sync.dma_start(b2s[:], b2d[:, :])
        ids = None
        if scales:
            ids = cpool.tile([128, len(scales) * 128], F16, tag="ident")
            nc.sync.dma_start(ids[:], idd[:, :])
        id8s = None
        if n8:
            id8s = cpool.tile([128, 128], F8, tag="ident8")
            nc.sync.dma_start(id8s[:], id8d[:, :])

        def ident(coeff):
            j = scales.index(coeff)
            return ids[:, j * 128:(j + 1) * 128]

        def body(rep):
            tt = tpool.tile([128, NCOL], F16, tag="term")
            nc.sync.dma_start(tt[:], termT[:, :])

            def ts(k, dt=None):
                if dt is None:
                    return tt[:, k * KC:(k + 1) * KC]
                return tt[:, k * KC + dt * BC:k * KC + (dt + 1) * BC]

            accs = [None] * T
            emb_tiles = {}

            def load_emb(k):
                g = groups[k]
                chunks = []
                idxs = g["emb"]
                for c0 in range(0, len(idxs), ECHUNK):
                    cnt = min(ECHUNK, len(idxs) - c0)
                    if use8[k]:
                        j0 = f8_pos[idxs[c0]]
                        et = e8pool.tile([128, ECHUNK * KC], F8, tag="e8")
                        nc.sync.dma_start(
                            et[:, :cnt * KC],
                            emb8[:, j0 * KC:(j0 + cnt) * KC])
                    else:
                        j0 = f16_pos[idxs[c0]]
                        et = e16pool.tile([128, ECHUNK * KC], F16, tag="e16")
                        nc.sync.dma_start(
                            et[:, :cnt * KC],
                            emb16[:, j0 * KC:(j0 + cnt) * KC])
                    chunks.append((et, cnt))
                emb_tiles[k] = chunks

            def agg(k):
                g = groups[k]
                eng_name = assign[k]
                a = apool.tile([128, KC], F16, tag=f"acc_{k}")
                accs[k] = a
                chunks = emb_tiles.pop(k, [])
                if eng_name == "pe":
                    ps = psapool.tile([128, KC], F32, tag="psa")
                    # per-dt PSUM regions with uniform [128, BC] extents
                    nops = 1 + len(g["pairs"]) + len(g["emb"])
                    for dt in range(DT):
                        n = 1
                        nc.tensor.matmul(
                            ps[:, dt * BC:(dt + 1) * BC],
                            ident(g["self_coeff"]), ts(k, dt),
                            start=True, stop=(n == nops))
                        for src, coeff in g["pairs"]:
                            n += 1
                            nc.tensor.matmul(
                                ps[:, dt * BC:(dt + 1) * BC],
                                ident(coeff), ts(src, dt),
                                start=False, stop=(n == nops))
                        j = 0
                        for et, cnt in chunks:
                            for i in range(cnt):
                                n += 1
                                nc.tensor.matmul(
                                    ps[:, dt * BC:(dt + 1) * BC],
                                    id8s[:, :],
                                    et[:, i * KC + dt * BC:
                                       i * KC + (dt + 1) * BC],
                                    start=False, stop=(n == nops))
                                j += 1
                    nc.scalar.activation(a[:], ps[:], AF.Copy,
                                         bias=0.0, scale=1.0)
                    return
                eng = nc.vector if eng_name == "dve" else nc.gpsimd
                nc.vector.tensor_scalar_mul(a[:], ts(k), g["self_coeff"])
                for src, coeff in g["pairs"]:
                    if coeff == 1.0:
                        eng.tensor_add(a[:], a[:], ts(src))
                    elif coeff == -1.0:
                        eng.tensor_sub(a[:], a[:], ts(src))
                    elif eng_name == "pool" and float(coeff).is_integer():
                        op = (eng.tensor_add if coeff > 0
                              else eng.tensor_sub)
                        for _ in range(int(abs(coeff))):
                            op(a[:], a[:], ts(src))
                    else:
                        # TensorScalarPtr is DVE-only
                        nc.vector.scalar_tensor_tensor(
                            a[:], ts(src), coeff, a[:], OP.mult, OP.add)
                for et, cnt in chunks:
                    for i in range(cnt):
                        eng.tensor_add(a[:], a[:],
                                       et[:, i * KC:(i + 1) * KC])

            epi_budget = plan["epi_dve"]

            def mlp(p):
                nonlocal epi_budget
                k0, k1 = 2 * p, 2 * p + 1
                hids = []
                for ht in range(HT):
                    ps = ps1pool.tile([128, 2 * BC], F32, tag="ps1")
                    for sub, k in ((0, k0), (1, k1)):
                        for dt in range(DT):
                            nc.tensor.matmul(
                                ps[:, sub * BC:(sub + 1) * BC],
                                w1s[:, dt * H + ht * 128:
                                    dt * H + (ht + 1) * 128],
                                accs[k][:, dt * BC:(dt + 1) * BC],
                                start=(dt == 0), stop=(dt == DT - 1))
                    hid = hpool.tile([128, 2 * BC], F16, tag="hid")
                    if epi_budget > 0:
                        epi_budget -= 1
                        nc.vector.tensor_scalar(
                            hid[:], ps[:], b1s[:, ht:ht + 1], 0.0,
                            OP.add, OP.max)
                    else:
                        nc.scalar.activation(hid[:], ps[:], AF.Relu,
                                             bias=b1s[:, ht:ht + 1], scale=1.0)
                    hids.append(hid)
                for dt2 in range(DT):
                    ps2 = ps2pool.tile([128, 2 * BC], F32, tag="ps2")
                    for sub in range(2):
                        for ht in range(HT):
                            nc.tensor.matmul(
                                ps2[:, sub * BC:(sub + 1) * BC],
                                w2s[:, ht * D + dt2 * 128:
                                    ht * D + (dt2 + 1) * 128],
                                hids[ht][:, sub * BC:(sub + 1) * BC],
                                start=(ht == 0), stop=(ht == HT - 1))
                    ot = opool.tile([128, 2 * BC], F16, tag="ot")
                    nc.scalar.activation(ot[:], ps2[:], AF.Identity,
                                         bias=b2s[:, dt2:dt2 + 1], scale=1.0)
                    nc.sync.dma_start(
                        outT[:, dt2 * T * BC + p * 2 * BC:
                             dt2 * T * BC + (p + 1) * 2 * BC], ot[:])

            # software pipeline: aggregate one pair ahead of the MLP so the
            # Act-engine merges / DVE adds for pair p+1 queue BEFORE pair p's
            # epilogue tiles and the PE stays fed.
            for k in range(4):
                load_emb(k)
            agg(0)
            agg(1)
            for p in range(T // 2):
                if 2 * p + 4 < T:
                    load_emb(2 * p + 4)
                if 2 * p + 5 < T:
                    load_emb(2 * p + 5)
                if 2 * p + 2 < T:
                    agg(2 * p + 2)
                    agg(2 * p + 3)
                mlp(p)

        if loop:
            with tc.For_i(0, loop, 1,
                          hint_engines=(ET.PE, ET.DVE, ET.Activation, ET.SP)):
                for rep in range(repeats):
                    body(rep)
        else:
            for rep in range(repeats):
                body(rep)

    nc.compile()
    _KERNEL_CACHE[key] = nc
    return nc


def _prep_inputs(term_emb, pred_emb, inv_pred_emb, W1, b1, W2, b2, msgs):
    """Shard/transpose/cast host-side into per-core device layouts."""
    import ml_dtypes
    plan = _plan(tuple(msgs))
    f16_order, f8_order = plan["f16_order"], plan["f8_order"]
    scales = plan["scales"]

    def fuse(a):
        # [X, BC, D] -> [128, X*KC] with column = x*KC + dt*BC + b
        X = a.shape[0]
        a2 = a.transpose(0, 2, 1).reshape(X, DT, 128, BC)
        return np.ascontiguousarray(
            a2.transpose(2, 0, 1, 3).reshape(128, X * KC))

    w1t = np.ascontiguousarray(
        W1.astype(np.float16).reshape(DT, 128, H).transpose(1, 0, 2)
        .reshape(128, DT * H))
    w2t = np.ascontiguousarray(
        W2.astype(np.float16).reshape(HT, 128, D).transpose(1, 0, 2)
        .reshape(128, HT * D))
    b1t = np.ascontiguousarray(b1.astype(np.float32).reshape(HT, 128).T)
    b2t = np.ascontiguousarray(b2.astype(np.float32).reshape(DT, 128).T)
    identT = np.zeros((128, max(len(scales), 1) * 128), np.float16)
    for j, c in enumerate(scales):
        identT[:, j * 128:(j + 1) * 128] = np.eye(128, dtype=np.float16) * c
    ident8 = np.ascontiguousarray(np.eye(128).astype(ml_dtypes.float8_e3m4))

    msg_arrs = []
    for (dst, src, s, which, e) in plan["msgs"]:
        arr = pred_emb if which == 0 else inv_pred_emb
        msg_arrs.append((arr, e, s))

    in_maps = []
    for c in range(N_CORES):
        sl = slice(c * BC, (c + 1) * BC)
        termTc = fuse(term_emb[:, sl, :].astype(np.float16))
        n16 = max(len(f16_order), 1)
        n8 = max(len(f8_order), 1)
        e16 = np.zeros((128, n16 * KC), np.float16)
        e8 = np.zeros((128, n8 * KC), ml_dtypes.float8_e3m4)
        for j, mi in enumerate(f16_order):
            arr, e, s = msg_arrs[mi]
            a = arr[e, sl, :][None].astype(np.float32)
            if s != 1.0:
                a = a * s
            e16[:, j * KC:(j + 1) * KC] = fuse(a.astype(np.float16))
        for j, mi in enumerate(f8_order):
            arr, e, s = msg_arrs[mi]
            a = arr[e, sl, :][None].astype(np.float32)
            if s != 1.0:
                a = a * s
            e8[:, j * KC:(j + 1) * KC] = fuse(a).astype(ml_dtypes.float8_e3m4)
        in_maps.append(dict(termT=termTc, emb16=e16, emb8=e8, w1t=w1t,
                            w2t=w2t, b1t=b1t, b2t=b2t, identT=identT,
                            ident8=ident8))
    return in_maps


def kernel(term_emb, pred_emb, inv_pred_emb, signs, W1, b1, W2, b2,
           heads, tails):
    term_emb = np.asarray(term_emb, dtype=np.float32)
    pred_emb = np.asarray(pred_emb, dtype=np.float32)
    inv_pred_emb = np.asarray(inv_pred_emb, dtype=np.float32)
    signs = np.asarray(signs, dtype=np.float32)
    W1 = np.asarray(W1, dtype=np.float32)
    b1 = np.asarray(b1, dtype=np.float32)
    W2 = np.asarray(W2, dtype=np.float32)
    b2 = np.asarray(b2, dtype=np.float32)
    heads = np.asarray(heads).astype(np.int64)
    tails = np.asarray(tails).astype(np.int64)

    msgs = _messages(heads, tails, signs)
    nc = _build(tuple(msgs))
    in_maps = _prep_inputs(term_emb, pred_emb, inv_pred_emb, W1, b1, W2, b2,
                           msgs)
    res = run_bass_kernel_spmd(nc, in_maps, list(range(N_CORES)))

    out = np.empty((T, B, D), np.float32)
    for c in range(N_CORES):
        o = res.results[c]["outT"].astype(np.float32)
        # o[p, dt*T*BC + k*BC + b] -> out[k, c*BC+b, dt*128+p]
        o4 = o.reshape(128, DT, T, BC).transpose(2, 3, 1, 0)  # [T, BC, DT, 128]
        out[:, c * BC:(c + 1) * BC, :] = o4.reshape(T, BC, D)
    return out


# revision 17
# speedup vs baseline: 1.1037x; 1.1037x over previous
"""Trainium2 Bass kernel for nn_LogicalGNNLayer (GNN message passing + MLP).

Computation (reference):
    h = term_emb[heads]; t = term_emb[tails]           # gather  [E,B,D]
    agg = segsum(s*(h+pred), tails) + segsum(s*(t+inv), heads)   # [T,B,D]
    agg += EPS*term_emb
    out = relu(agg @ W1 + b1) @ W2 + b2                # [T,B,D]

Strategy (v1 rewrite):
  - Batch B sharded across 8 cores (Bc=512); term/edge structure and weights
    replicated.  heads/tails/signs are read on the host and the message
    structure is baked into the program.
  - Fused on-chip layout: 128 partitions hold d-within-tile, the free axis
    holds (k, dt, b) for terms (so each term/acc is one contiguous
    [128, 1024] span) and (dt, k, b) for the output.
  - The aggregation acc[k] = (EPS + selfs)*term[k] + sum c*term[src] + sum emb
    is elementwise work balanced across THREE engines (HW-calibrated costs):
      * DVE:  tensor ops at 2x fp16 mode (~716ns per [128,1024] add)
      * Pool: tensor ops (~2.08us/add; slow but otherwise idle)
      * PE:   scaled-identity matmuls accumulating in PSUM (~174ns/512 cols),
              merged to SBUF by one Act-engine copy per group
  - emb tensors for PE/Pool groups are cast to fp8e3 (e3m4) on the host,
    halving their DMA traffic at no engine cost; DVE groups keep fp16 (fp8
    operands drop DVE to 1x mode).  Per-group dtype is a balance knob.
  - MLP: fp16 matmuls with fp32 PSUM accumulation; ReLU+b1 / +b2 epilogues
    on the Act engine via bias APs (nonzero biases handled), with an
    adjustable DVE share.
  - Output stored fp16 (cast to fp32 on the host).
"""

import numpy as np

import concourse.bass as bass
import concourse.tile as tile
from concourse import bacc, mybir
from concourse.bass_utils import run_bass_kernel_spmd

T, B, D, H, E = 16, 4096, 256, 512, 32
EPS = 0.1
N_CORES = 8
BC = B // N_CORES            # 512 batch per core
DT = D // 128                # 2 d-tiles
HT = H // 128                # 4 h-tiles
KC = DT * BC                 # 1024 columns per term (dt-fused)
NCOL = T * KC                # 16384 free-axis columns
ECHUNK = 8                   # messages per emb DMA/tile
F16 = mybir.dt.float16
F32 = mybir.dt.float32
F8 = mybir.dt.float8e3       # e3m4: 4 mantissa bits
AF = mybir.ActivationFunctionType
OP = mybir.AluOpType
ET = mybir.EngineType

# HW-calibrated per-op costs in us ([128,1024]-equivalent; microbenched on
# the axon-tunneled trn2):
C_DVE_ADD = 0.716
C_DVE_ADD8 = 1.295
C_DVE_STT = 1.35
C_DVE_INIT = 0.39
C_DVE_EPI = 1.43
C_POOL_ADD = 2.08
C_POOL_STT = 1.55
C_PE_512 = 0.174
C_ACT_TILE = 1.04
C_MLP = 44.5
C_ACT_FIXED = 48 * 1.037     # 32 relu + 16 out tiles, all on Act
US_PER_MIB = 2.697
DMA_FIXED_MIB = 4.0 + 4.0 + 0.625 + 0.07   # term + out + weights + ident
C_SP_DMA = 0.565

_KERNEL_CACHE = {}


def _messages(heads, tails, signs):
    """Directed message list (dst, src, sign, which_emb, e)."""
    msgs = []
    for e in range(E):
        h, t, s = int(heads[e]), int(tails[e]), float(signs[e])
        assert 0 <= h < T and 0 <= t < T
        msgs.append((t, h, s, 0, e))   # msg_to_tail: acc[t] += s*(term[h]+pred[e])
        msgs.append((h, t, s, 1, e))   # msg_to_head: acc[h] += s*(term[t]+inv[e])
    return msgs


def _plan(msgs_key):
    """Group messages by destination, assign each group to an engine, lay
    out the emb DRAM ordering, and pick the epilogue split — all from
    HW-calibrated cost estimates."""
    msgs = list(msgs_key)
    groups = []
    for k in range(T):
        mlist = [(i, m) for i, m in enumerate(msgs) if m[0] == k]
        self_coeff = EPS + sum(m[2] for _, m in mlist if m[1] == k)
        pair_coeffs = {}
        for _, (_d, src, s, _w, _e) in mlist:
            if src != k:
                pair_coeffs[src] = pair_coeffs.get(src, 0.0) + s
        pairs = [(src, c) for src, c in sorted(pair_coeffs.items()) if c != 0.0]
        emb = [i for i, _m in mlist]   # every message carries its emb term
        groups.append(dict(k=k, self_coeff=self_coeff, pairs=pairs, emb=emb))

    def dve_cost(g):
        c = sum(C_DVE_ADD if abs(co) == 1.0 else C_DVE_STT
                for _s, co in g["pairs"])
        return c + len(g["emb"]) * C_DVE_ADD

    def pool_cost(g):
        # Pool lacks TensorScalarPtr: |coeff|==c emitted as c adds/subs
        c = sum(abs(co) * C_POOL_ADD if float(co).is_integer() else C_DVE_STT
                for _s, co in g["pairs"])
        return c + len(g["emb"]) * C_POOL_ADD

    def pe_cost(g):
        return (2 + 2 * len(g["pairs"]) + 2 * len(g["emb"])) * C_PE_512

    est = {"pe": C_MLP, "dve": T * C_DVE_INIT, "pool": 0.0,
           "act": C_ACT_FIXED, "dma": 0.0, "sp": 40 * C_SP_DMA}
    emb_mib = 0.0
    assign = {}
    order = sorted(range(T), key=lambda k: -dve_cost(groups[k]))
    for k in order:
        g = groups[k]
        nmsg = len(g["emb"])
        dma16 = (DMA_FIXED_MIB + emb_mib + 0.25 * nmsg) * US_PER_MIB
        dma8 = (DMA_FIXED_MIB + emb_mib + 0.125 * nmsg) * US_PER_MIB
        cand = {
            "dve": max(est["pe"], est["dve"] + dve_cost(g), est["pool"],
                       est["act"], dma16),
            # Pool reading fp8 crashes the device (NRT_EXEC_UNIT_UNRECOVERABLE)
            # so pool groups keep fp16 emb.
            "pool": max(est["pe"], est["dve"], est["pool"] + pool_cost(g),
                        est["act"], dma16),
            "pe": max(est["pe"] + pe_cost(g), est["dve"] - C_DVE_INIT,
                      est["pool"], est["act"] + C_ACT_TILE, dma8),
        }
        eng = min(cand, key=lambda e: cand[e])
        assign[k] = eng
        if eng == "dve":
            est["dve"] += dve_cost(g)
            emb_mib += 0.25 * nmsg
        elif eng == "pool":
            est["pool"] += pool_cost(g)
            emb_mib += 0.25 * nmsg
        else:
            est["pe"] += pe_cost(g)
            est["act"] += C_ACT_TILE
            est["dve"] -= C_DVE_INIT
            emb_mib += 0.125 * nmsg
    est["dma"] = (DMA_FIXED_MIB + emb_mib) * US_PER_MIB

    # epilogue split: move tiles Act -> DVE while the makespan improves
    epi_dve = 0
    while epi_dve < 24:
        cur = max(est.values())
        nxt_act, nxt_dve = est["act"] - 1.037, est["dve"] + C_DVE_EPI
        if max(est["pe"], est["pool"], est["dma"], est["sp"],
               nxt_act, nxt_dve) < cur:
            est["act"], est["dve"] = nxt_act, nxt_dve
            epi_dve += 1
        else:
            break
    epi_dve = max(epi_dve, 4)   # HW: Act queueing exceeds the model

    # flip DVE-group emb to fp8 while DMA dominates (NOT pool: fp8 on the
    # Pool engine crashes the device)
    use8 = {k: assign[k] == "pe" for k in range(T)}
    for k in order:
        if assign[k] != "dve":
            continue
        nmsg = len(groups[k]["emb"])
        if nmsg and est["dma"] == max(est.values()):
            nxt_dve = est["dve"] + (C_DVE_ADD8 - C_DVE_ADD) * nmsg
            nxt_dma = est["dma"] - 0.125 * nmsg * US_PER_MIB
            if max(nxt_dve, nxt_dma) < max(est["dve"], est["dma"]):
                est["dve"], est["dma"] = nxt_dve, nxt_dma
                use8[k] = True

    # emb DRAM layouts: grouped by k in processing order
    f16_order, f8_order = [], []
    for k in range(T):
        (f8_order if use8[k] else f16_order).extend(groups[k]["emb"])

    scales = []
    for k in range(T):
        if assign[k] != "pe":
            continue
        g = groups[k]
        for c in [g["self_coeff"]] + [c for _s, c in g["pairs"]]:
            if c not in scales:
                scales.append(c)

    return dict(groups=groups, assign=assign, use8=use8,
                f16_order=f16_order, f8_order=f8_order, scales=scales,
                epi_dve=epi_dve, est=est, msgs=msgs)


def _build(msgs_key, repeats=1, loop=0):
    key = (msgs_key, repeats, loop)
    if key in _KERNEL_CACHE:
        return _KERNEL_CACHE[key]
    plan = _plan(msgs_key)
    groups, assign, use8 = plan["groups"], plan["assign"], plan["use8"]
    f16_pos = {m: j for j, m in enumerate(plan["f16_order"])}
    f8_pos = {m: j for j, m in enumerate(plan["f8_order"])}
    scales = plan["scales"]
    n16, n8 = len(plan["f16_order"]), len(plan["f8_order"])

    nc = bacc.Bacc("TRN2", target_bir_lowering=False, debug=False,
                   num_devices=N_CORES)
    termT = nc.declare_dram_parameter("termT", [128, NCOL], F16, isOutput=False)
    emb16 = nc.declare_dram_parameter("emb16", [128, max(n16, 1) * KC], F16,
                                      isOutput=False)
    emb8 = nc.declare_dram_parameter("emb8", [128, max(n8, 1) * KC], F8,
                                     isOutput=False)
    w1d = nc.declare_dram_parameter("w1t", [128, DT * H], F16, isOutput=False)
    w2d = nc.declare_dram_parameter("w2t", [128, HT * D], F16, isOutput=False)
    b1d = nc.declare_dram_parameter("b1t", [128, HT], F32, isOutput=False)
    b2d = nc.declare_dram_parameter("b2t", [128, DT], F32, isOutput=False)
    idd = nc.declare_dram_parameter("identT", [128, max(len(scales), 1) * 128],
                                    F16, isOutput=False)
    id8d = nc.declare_dram_parameter("ident8", [128, 128], F8, isOutput=False)
    outT = nc.declare_dram_parameter("outT", [128, NCOL], F16, isOutput=True)

    with nc.allow_low_precision(reason="fp16/fp8 on-chip"), \
            tile.TileContext(nc) as tc, \
            tc.tile_pool(name="const", bufs=1) as cpool, \
            tc.tile_pool(name="term", bufs=2) as tpool, \
            tc.tile_pool(name="acc", bufs=1) as apool, \
            tc.tile_pool(name="e16p", bufs=4) as e16pool, \
            tc.tile_pool(name="e8p", bufs=2) as e8pool, \
            tc.tile_pool(name="hid", bufs=8) as hpool, \
            tc.tile_pool(name="out", bufs=4) as opool, \
            tc.tile_pool(name="ps1", bufs=2, space="PSUM") as ps1pool, \
            tc.tile_pool(name="ps2", bufs=1, space="PSUM") as ps2pool, \
            tc.tile_pool(name="psa", bufs=1, space="PSUM") as psapool:

        # ---- persistent loads -------------------------------------------
        w1s = cpool.tile([128, DT * H], F16, tag="w1")
        nc.sync.dma_start(w1s[:], w1d[:, :])
        w2s = cpool.tile([128, HT * D], F16, tag="w2")
        nc.sync.dma_start(w2s[:], w2d[:, :])
        b1s = cpool.tile([128, HT], F32, tag="b1")
        nc.sync.dma_start(b1s[:], b1d[:, :])
        b2s = cpool.tile([128, DT], F32, tag="b2")
        nc.sync.dma_start(b2s[:], b2d[:, :])
        ids = None
        if scales:
            ids = cpool.tile([128, len(scales) * 128], F16, tag="ident")
            nc.sync.dma_start(ids[:], idd[:, :])
        id8s = None
        if n8:
            id8s = cpool.tile([128, 128], F8, tag="ident8")
            nc.sync.dma_start(id8s[:], id8d[:, :])

        def ident(coeff):
            j = scales.index(coeff)
            return ids[:, j * 128:(j + 1) * 128]

        def body(rep):
            tt = tpool.tile([128, NCOL], F16, tag="term")
            nc.sync.dma_start(tt[:], termT[:, :])

            def ts(k, dt=None):
                if dt is None:
                    return tt[:, k * KC:(k + 1) * KC]
                return tt[:, k * KC + dt * BC:k * KC + (dt + 1) * BC]

            accs = [None] * T
            emb_tiles = {}

            def load_emb(k):
                g = groups[k]
                chunks = []
                idxs = g["emb"]
                for c0 in range(0, len(idxs), ECHUNK):
                    cnt = min(ECHUNK, len(idxs) - c0)
                    if use8[k]:
                        j0 = f8_pos[idxs[c0]]
                        et = e8pool.tile([128, ECHUNK * KC], F8, tag="e8")
                        nc.sync.dma_start(
                            et[:, :cnt * KC],
                            emb8[:, j0 * KC:(j0 + cnt) * KC])
                    else:
                        j0 = f16_pos[idxs[c0]]
                        et = e16pool.tile([128, ECHUNK * KC], F16, tag="e16")
                        nc.sync.dma_start(
                            et[:, :cnt * KC],
                            emb16[:, j0 * KC:(j0 + cnt) * KC])
                    chunks.append((et, cnt))
                emb_tiles[k] = chunks

            def agg(k):
                g = groups[k]
                eng_name = assign[k]
                a = apool.tile([128, KC], F16, tag=f"acc_{k}")
                accs[k] = a
                chunks = emb_tiles.pop(k, [])
                if eng_name == "pe":
                    ps = psapool.tile([128, KC], F32, tag="psa")
                    # per-dt PSUM regions with uniform [128, BC] extents
                    nops = 1 + len(g["pairs"]) + len(g["emb"])
                    for dt in range(DT):
                        n = 1
                        nc.tensor.matmul(
                            ps[:, dt * BC:(dt + 1) * BC],
                            ident(g["self_coeff"]), ts(k, dt),
                            start=True, stop=(n == nops))
                        for src, coeff in g["pairs"]:
                            n += 1
                            nc.tensor.matmul(
                                ps[:, dt * BC:(dt + 1) * BC],
                                ident(coeff), ts(src, dt),
                                start=False, stop=(n == nops))
                        j = 0
                        for et, cnt in chunks:
                            for i in range(cnt):
                                n += 1
                                nc.tensor.matmul(
                                    ps[:, dt * BC:(dt + 1) * BC],
                                    id8s[:, :],
                                    et[:, i * KC + dt * BC:
                                       i * KC + (dt + 1) * BC],
                                    start=False, stop=(n == nops))
                                j += 1
                    nc.scalar.activation(a[:], ps[:], AF.Copy,
                                         bias=0.0, scale=1.0)
                    return
                eng = nc.vector if eng_name == "dve" else nc.gpsimd
                nc.vector.tensor_scalar_mul(a[:], ts(k), g["self_coeff"])
                for src, coeff in g["pairs"]:
                    if coeff == 1.0:
                        eng.tensor_add(a[:], a[:], ts(src))
                    elif coeff == -1.0:
                        eng.tensor_sub(a[:], a[:], ts(src))
                    elif eng_name == "pool" and float(coeff).is_integer():
                        op = (eng.tensor_add if coeff > 0
                              else eng.tensor_sub)
                        for _ in range(int(abs(coeff))):
                            op(a[:], a[:], ts(src))
                    else:
                        # TensorScalarPtr is DVE-only
                        nc.vector.scalar_tensor_tensor(
                            a[:], ts(src), coeff, a[:], OP.mult, OP.add)
                for et, cnt in chunks:
                    for i in range(cnt):
                        eng.tensor_add(a[:], a[:],
                                       et[:, i * KC:(i + 1) * KC])

            n_epi = plan["epi_dve"]
            dve_relu = set()
            if n_epi:
                step = max(1, (4 * (T // 2)) // n_epi)
                dve_relu = {(i // HT, i % HT)
                            for i in range(0, 4 * (T // 2), step)}

            def mlp(p):
                k0, k1 = 2 * p, 2 * p + 1
                hids = []
                for ht in range(HT):
                    ps = ps1pool.tile([128, 2 * BC], F32, tag="ps1")
                    for sub, k in ((0, k0), (1, k1)):
                        for dt in range(DT):
                            nc.tensor.matmul(
                                ps[:, sub * BC:(sub + 1) * BC],
                                w1s[:, dt * H + ht * 128:
                                    dt * H + (ht + 1) * 128],
                                accs[k][:, dt * BC:(dt + 1) * BC],
                                start=(dt == 0), stop=(dt == DT - 1))
                    hid = hpool.tile([128, 2 * BC], F16, tag="hid")
                    if (p, ht) in dve_relu:
                        nc.vector.tensor_scalar(
                            hid[:], ps[:], b1s[:, ht:ht + 1], 0.0,
                            OP.add, OP.max)
                    else:
                        nc.scalar.activation(hid[:], ps[:], AF.Relu,
                                             bias=b1s[:, ht:ht + 1], scale=1.0)
                    hids.append(hid)
                for dt2 in range(DT):
                    ps2 = ps2pool.tile([128, 2 * BC], F32, tag="ps2")
                    for sub in range(2):
                        for ht in range(HT):
                            nc.tensor.matmul(
                                ps2[:, sub * BC:(sub + 1) * BC],
                                w2s[:, ht * D + dt2 * 128:
                                    ht * D + (dt2 + 1) * 128],
                                hids[ht][:, sub * BC:(sub + 1) * BC],
                                start=(ht == 0), stop=(ht == HT - 1))
                    ot = opool.tile([128, 2 * BC], F16, tag="ot")
                    nc.scalar.activation(ot[:], ps2[:], AF.Identity,
                                         bias=b2s[:, dt2:dt2 + 1], scale=1.0)
                    nc.sync.dma_start(
                        outT[:, dt2 * T * BC + p * 2 * BC:
                             dt2 * T * BC + (p + 1) * 2 * BC], ot[:])

            # software pipeline: aggregate two pairs ahead of the MLP so the
            # Act-engine merges / DVE adds queue BEFORE older epilogue tiles
            # and the PE stays fed.
            for k in range(6):
                load_emb(k)
            for k in range(4):
                agg(k)
            for p in range(T // 2):
                if 2 * p + 6 < T:
                    load_emb(2 * p + 6)
                if 2 * p + 7 < T:
                    load_emb(2 * p + 7)
                if 2 * p + 4 < T:
                    agg(2 * p + 4)
                    agg(2 * p + 5)
                mlp(p)

        if loop:
            with tc.For_i(0, loop, 1,
                          hint_engines=(ET.PE, ET.DVE, ET.Activation, ET.SP)):
                for rep in range(repeats):
                    body(rep)
        else:
            for rep in range(repeats):
                body(rep)

    nc.compile()
    _KERNEL_CACHE[key] = nc
    return nc


def _prep_inputs(term_emb, pred_emb, inv_pred_emb, W1, b1, W2, b2, msgs):
    """Shard/transpose/cast host-side into per-core device layouts."""
    import ml_dtypes
    plan = _plan(tuple(msgs))
    f16_order, f8_order = plan["f16_order"], plan["f8_order"]
    scales = plan["scales"]

    def fuse(a):
        # [X, BC, D] -> [128, X*KC] with column = x*KC + dt*BC + b
        X = a.shape[0]
        a2 = a.transpose(0, 2, 1).reshape(X, DT, 128, BC)
        return np.ascontiguousarray(
            a2.transpose(2, 0, 1, 3).reshape(128, X * KC))

    w1t = np.ascontiguousarray(
        W1.astype(np.float16).reshape(DT, 128, H).transpose(1, 0, 2)
        .reshape(128, DT * H))
    w2t = np.ascontiguousarray(
        W2.astype(np.float16).reshape(HT, 128, D).transpose(1, 0, 2)
        .reshape(128, HT * D))
    b1t = np.ascontiguousarray(b1.astype(np.float32).reshape(HT, 128).T)
    b2t = np.ascontiguousarray(b2.astype(np.float32).reshape(DT, 128).T)
    identT = np.zeros((128, max(len(scales), 1) * 128), np.float16)
    for j, c in enumerate(scales):
        identT[:, j * 128:(j + 1) * 128] = np.eye(128, dtype=np.float16) * c
    ident8 = np.ascontiguousarray(np.eye(128).astype(ml_dtypes.float8_e3m4))

    msg_arrs = []
    for (dst, src, s, which, e) in plan["msgs"]:
        arr = pred_emb if which == 0 else inv_pred_emb
        msg_arrs.append((arr, e, s))

    in_maps = []
    for c in range(N_CORES):
        sl = slice(c * BC, (c + 1) * BC)
        termTc = fuse(term_emb[:, sl, :].astype(np.float16))
        n16 = max(len(f16_order), 1)
        n8 = max(len(f8_order), 1)
        e16 = np.zeros((128, n16 * KC), np.float16)
        e8 = np.zeros((128, n8 * KC), ml_dtypes.float8_e3m4)
        for j, mi in enumerate(f16_order):
            arr, e, s = msg_arrs[mi]
            a = arr[e, sl, :][None].astype(np.float32)
            if s != 1.0:
                a = a * s
            e16[:, j * KC:(j + 1) * KC] = fuse(a.astype(np.float16))
        for j, mi in enumerate(f8_order):
            arr, e, s = msg_arrs[mi]
            a = arr[e, sl, :][None].astype(np.float32)
            if s != 1.0:
                a = a * s
            e8[:, j * KC:(j + 1) * KC] = fuse(a).astype(ml_dtypes.float8_e3m4)
        in_maps.append(dict(termT=termTc, emb16=e16, emb8=e8, w1t=w1t,
                            w2t=w2t, b1t=b1t, b2t=b2t, identT=identT,
                            ident8=ident8))
    return in_maps


def kernel(term_emb, pred_emb, inv_pred_emb, signs, W1, b1, W2, b2,
           heads, tails):
    term_emb = np.asarray(term_emb, dtype=np.float32)
    pred_emb = np.asarray(pred_emb, dtype=np.float32)
    inv_pred_emb = np.asarray(inv_pred_emb, dtype=np.float32)
    signs = np.asarray(signs, dtype=np.float32)
    W1 = np.asarray(W1, dtype=np.float32)
    b1 = np.asarray(b1, dtype=np.float32)
    W2 = np.asarray(W2, dtype=np.float32)
    b2 = np.asarray(b2, dtype=np.float32)
    heads = np.asarray(heads).astype(np.int64)
    tails = np.asarray(tails).astype(np.int64)

    msgs = _messages(heads, tails, signs)
    nc = _build(tuple(msgs))
    in_maps = _prep_inputs(term_emb, pred_emb, inv_pred_emb, W1, b1, W2, b2,
                           msgs)
    res = run_bass_kernel_spmd(nc, in_maps, list(range(N_CORES)))

    out = np.empty((T, B, D), np.float32)
    for c in range(N_CORES):
        o = res.results[c]["outT"].astype(np.float32)
        # o[p, dt*T*BC + k*BC + b] -> out[k, c*BC+b, dt*128+p]
        o4 = o.reshape(128, DT, T, BC).transpose(2, 3, 1, 0)  # [T, BC, DT, 128]
        out[:, c * BC:(c + 1) * BC, :] = o4.reshape(T, BC, D)
    return out


# revision 20
# speedup vs baseline: 1.1532x; 1.0448x over previous
"""Trainium2 Bass kernel for nn_LogicalGNNLayer (GNN message passing + MLP).

Computation (reference):
    h = term_emb[heads]; t = term_emb[tails]           # gather  [E,B,D]
    agg = segsum(s*(h+pred), tails) + segsum(s*(t+inv), heads)   # [T,B,D]
    agg += EPS*term_emb
    out = relu(agg @ W1 + b1) @ W2 + b2                # [T,B,D]

Strategy (v1 rewrite):
  - Batch B sharded across 8 cores (Bc=512); term/edge structure and weights
    replicated.  heads/tails/signs are read on the host and the message
    structure is baked into the program.
  - Fused on-chip layout: 128 partitions hold d-within-tile, the free axis
    holds (k, dt, b) for terms (so each term/acc is one contiguous
    [128, 1024] span) and (dt, k, b) for the output.
  - The aggregation acc[k] = (EPS + selfs)*term[k] + sum c*term[src] + sum emb
    is elementwise work balanced across THREE engines (HW-calibrated costs):
      * DVE:  tensor ops at 2x fp16 mode (~716ns per [128,1024] add)
      * Pool: tensor ops (~2.08us/add; slow but otherwise idle)
      * PE:   scaled-identity matmuls accumulating in PSUM (~174ns/512 cols),
              merged to SBUF by one Act-engine copy per group
  - emb tensors for PE/Pool groups are cast to fp8e3 (e3m4) on the host,
    halving their DMA traffic at no engine cost; DVE groups keep fp16 (fp8
    operands drop DVE to 1x mode).  Per-group dtype is a balance knob.
  - MLP: fp16 matmuls with fp32 PSUM accumulation; ReLU+b1 / +b2 epilogues
    on the Act engine via bias APs (nonzero biases handled), with an
    adjustable DVE share.
  - Output stored fp16 (cast to fp32 on the host).
"""

import numpy as np

import concourse.bass as bass
import concourse.tile as tile
from concourse import bacc, mybir
from concourse.bass_utils import run_bass_kernel_spmd

T, B, D, H, E = 16, 4096, 256, 512, 32
EPS = 0.1
N_CORES = 8
BC = B // N_CORES            # 512 batch per core
DT = D // 128                # 2 d-tiles
HT = H // 128                # 4 h-tiles
KC = DT * BC                 # 1024 columns per term (dt-fused)
NCOL = T * KC                # 16384 free-axis columns
ECHUNK = 8                   # messages per emb DMA/tile
F16 = mybir.dt.float16
F32 = mybir.dt.float32
F8 = mybir.dt.float8e3       # e3m4: 4 mantissa bits
AF = mybir.ActivationFunctionType
OP = mybir.AluOpType
ET = mybir.EngineType

# HW-calibrated per-op costs in us ([128,1024]-equivalent; microbenched on
# the axon-tunneled trn2):
C_DVE_ADD = 0.716
C_DVE_ADD8 = 1.295
C_DVE_STT = 1.35
C_DVE_INIT = 0.39
C_DVE_EPI = 1.43
C_POOL_ADD = 2.08
C_POOL_STT = 1.55
C_PE_512 = 0.174
C_ACT_TILE = 1.04
C_MLP = 44.5
C_ACT_FIXED = 48 * 1.037     # 32 relu + 16 out tiles, all on Act
US_PER_MIB = 2.697
DMA_FIXED_MIB = 4.0 + 4.0 + 0.625 + 0.07   # term + out + weights + ident
C_SP_DMA = 0.565

_KERNEL_CACHE = {}


def _messages(heads, tails, signs):
    """Directed message list (dst, src, sign, which_emb, e)."""
    msgs = []
    for e in range(E):
        h, t, s = int(heads[e]), int(tails[e]), float(signs[e])
        assert 0 <= h < T and 0 <= t < T
        msgs.append((t, h, s, 0, e))   # msg_to_tail: acc[t] += s*(term[h]+pred[e])
        msgs.append((h, t, s, 1, e))   # msg_to_head: acc[h] += s*(term[t]+inv[e])
    return msgs


def _plan(msgs_key):
    """Group messages by destination, assign each group to an engine, lay
    out the emb DRAM ordering, and pick the epilogue split — all from
    HW-calibrated cost estimates."""
    msgs = list(msgs_key)
    groups = []
    for k in range(T):
        mlist = [(i, m) for i, m in enumerate(msgs) if m[0] == k]
        self_coeff = EPS + sum(m[2] for _, m in mlist if m[1] == k)
        pair_coeffs = {}
        for _, (_d, src, s, _w, _e) in mlist:
            if src != k:
                pair_coeffs[src] = pair_coeffs.get(src, 0.0) + s
        pairs = [(src, c) for src, c in sorted(pair_coeffs.items()) if c != 0.0]
        emb = [i for i, _m in mlist]   # every message carries its emb term
        groups.append(dict(k=k, self_coeff=self_coeff, pairs=pairs, emb=emb))

    def dve_cost(g):
        c = sum(C_DVE_ADD if abs(co) == 1.0 else C_DVE_STT
                for _s, co in g["pairs"])
        return c + len(g["emb"]) * C_DVE_ADD

    def pool_cost(g):
        # Pool lacks TensorScalarPtr: |coeff|==c emitted as c adds/subs
        c = sum(abs(co) * C_POOL_ADD if float(co).is_integer() else C_DVE_STT
                for _s, co in g["pairs"])
        return c + len(g["emb"]) * C_POOL_ADD

    def pe_cost(g):
        return (2 + 2 * len(g["pairs"]) + 2 * len(g["emb"])) * C_PE_512

    est = {"pe": C_MLP, "dve": T * C_DVE_INIT, "pool": 0.0,
           "act": C_ACT_FIXED, "dma": 0.0, "sp": 40 * C_SP_DMA}
    emb_mib = 0.0
    assign = {}
    order = sorted(range(T), key=lambda k: -dve_cost(groups[k]))
    for k in order:
        g = groups[k]
        nmsg = len(g["emb"])
        dma16 = (DMA_FIXED_MIB + emb_mib + 0.25 * nmsg) * US_PER_MIB
        dma8 = (DMA_FIXED_MIB + emb_mib + 0.125 * nmsg) * US_PER_MIB
        cand = {
            "dve": max(est["pe"], est["dve"] + dve_cost(g), est["pool"],
                       est["act"], dma16),
            # Pool reading fp8 crashes the device (NRT_EXEC_UNIT_UNRECOVERABLE)
            # so pool groups keep fp16 emb.
            "pool": max(est["pe"], est["dve"], est["pool"] + pool_cost(g),
                        est["act"], dma16),
            "pe": max(est["pe"] + pe_cost(g), est["dve"] - C_DVE_INIT,
                      est["pool"], est["act"] + C_ACT_TILE, dma8),
        }
        eng = min(cand, key=lambda e: cand[e])
        assign[k] = eng
        if eng == "dve":
            est["dve"] += dve_cost(g)
            emb_mib += 0.25 * nmsg
        elif eng == "pool":
            est["pool"] += pool_cost(g)
            emb_mib += 0.25 * nmsg
        else:
            est["pe"] += pe_cost(g)
            est["act"] += C_ACT_TILE
            est["dve"] -= C_DVE_INIT
            emb_mib += 0.125 * nmsg
    est["dma"] = (DMA_FIXED_MIB + emb_mib) * US_PER_MIB

    # epilogue split: move tiles Act -> DVE while the makespan improves
    epi_dve = 0
    while epi_dve < 24:
        cur = max(est.values())
        nxt_act, nxt_dve = est["act"] - 1.037, est["dve"] + C_DVE_EPI
        if max(est["pe"], est["pool"], est["dma"], est["sp"],
               nxt_act, nxt_dve) < cur:
            est["act"], est["dve"] = nxt_act, nxt_dve
            epi_dve += 1
        else:
            break

    # flip DVE-group emb to fp8 while DMA dominates (NOT pool: fp8 on the
    # Pool engine crashes the device)
    use8 = {k: assign[k] == "pe" for k in range(T)}
    for k in order:
        if assign[k] != "dve":
            continue
        nmsg = len(groups[k]["emb"])
        if nmsg and est["dma"] == max(est.values()):
            nxt_dve = est["dve"] + (C_DVE_ADD8 - C_DVE_ADD) * nmsg
            nxt_dma = est["dma"] - 0.125 * nmsg * US_PER_MIB
            if max(nxt_dve, nxt_dma) < max(est["dve"], est["dma"]):
                est["dve"], est["dma"] = nxt_dve, nxt_dma
                use8[k] = True

    # emb DRAM layouts: grouped by k in processing order
    f16_order, f8_order = [], []
    for k in range(T):
        (f8_order if use8[k] else f16_order).extend(groups[k]["emb"])

    scales = []
    for k in range(T):
        if assign[k] != "pe":
            continue
        g = groups[k]
        for c in [g["self_coeff"]] + [c for _s, c in g["pairs"]]:
            if c not in scales:
                scales.append(c)

    return dict(groups=groups, assign=assign, use8=use8,
                f16_order=f16_order, f8_order=f8_order, scales=scales,
                epi_dve=epi_dve, est=est, msgs=msgs)


def _build(msgs_key, repeats=1, loop=0):
    key = (msgs_key, repeats, loop)
    if key in _KERNEL_CACHE:
        return _KERNEL_CACHE[key]
    plan = _plan(msgs_key)
    groups, assign, use8 = plan["groups"], plan["assign"], plan["use8"]
    f16_pos = {m: j for j, m in enumerate(plan["f16_order"])}
    f8_pos = {m: j for j, m in enumerate(plan["f8_order"])}
    scales = plan["scales"]
    n16, n8 = len(plan["f16_order"]), len(plan["f8_order"])

    nc = bacc.Bacc("TRN2", target_bir_lowering=False, debug=False,
                   num_devices=N_CORES)
    termT = nc.declare_dram_parameter("termT", [128, NCOL], F16, isOutput=False)
    emb16 = nc.declare_dram_parameter("emb16", [128, max(n16, 1) * KC], F16,
                                      isOutput=False)
    emb8 = nc.declare_dram_parameter("emb8", [128, max(n8, 1) * KC], F8,
                                     isOutput=False)
    w1d = nc.declare_dram_parameter("w1t", [128, DT * H], F16, isOutput=False)
    w2d = nc.declare_dram_parameter("w2t", [128, HT * D], F16, isOutput=False)
    b1d = nc.declare_dram_parameter("b1t", [128, HT], F32, isOutput=False)
    b2d = nc.declare_dram_parameter("b2t", [128, DT], F32, isOutput=False)
    idd = nc.declare_dram_parameter("identT", [128, max(len(scales), 1) * 128],
                                    F16, isOutput=False)
    id8d = nc.declare_dram_parameter("ident8", [128, 128], F8, isOutput=False)
    outT = nc.declare_dram_parameter("outT", [128, NCOL], F16, isOutput=True)

    with nc.allow_low_precision(reason="fp16/fp8 on-chip"), \
            tile.TileContext(nc) as tc, \
            tc.tile_pool(name="const", bufs=1) as cpool, \
            tc.tile_pool(name="term", bufs=2) as tpool, \
            tc.tile_pool(name="acc", bufs=1) as apool, \
            tc.tile_pool(name="e16p", bufs=3) as e16pool, \
            tc.tile_pool(name="e8p", bufs=2) as e8pool, \
            tc.tile_pool(name="hid", bufs=8) as hpool, \
            tc.tile_pool(name="out", bufs=4) as opool, \
            tc.tile_pool(name="ps1", bufs=2, space="PSUM") as ps1pool, \
            tc.tile_pool(name="ps2", bufs=1, space="PSUM") as ps2pool, \
            tc.tile_pool(name="psa", bufs=1, space="PSUM") as psapool:

        # ---- persistent loads -------------------------------------------
        w1s = cpool.tile([128, DT * H], F16, tag="w1")
        nc.sync.dma_start(w1s[:], w1d[:, :])
        w2s = cpool.tile([128, HT * D], F16, tag="w2")
        nc.sync.dma_start(w2s[:], w2d[:, :])
        b1s = cpool.tile([128, HT], F32, tag="b1")
        nc.sync.dma_start(b1s[:], b1d[:, :])
        b2s = cpool.tile([128, DT], F32, tag="b2")
        nc.sync.dma_start(b2s[:], b2d[:, :])
        ids = None
        if scales:
            ids = cpool.tile([128, len(scales) * 128], F16, tag="ident")
            nc.sync.dma_start(ids[:], idd[:, :])
        id8s = None
        if n8:
            id8s = cpool.tile([128, 128], F8, tag="ident8")
            nc.sync.dma_start(id8s[:], id8d[:, :])

        def ident(coeff):
            j = scales.index(coeff)
            return ids[:, j * 128:(j + 1) * 128]

        def body(rep):
            tt = tpool.tile([128, NCOL], F16, tag="term")
            nc.sync.dma_start(tt[:], termT[:, :])

            def ts(k, dt=None):
                if dt is None:
                    return tt[:, k * KC:(k + 1) * KC]
                return tt[:, k * KC + dt * BC:k * KC + (dt + 1) * BC]

            accs = [None] * T
            emb_tiles = {}

            def load_emb(k):
                g = groups[k]
                chunks = []
                idxs = g["emb"]
                for c0 in range(0, len(idxs), ECHUNK):
                    cnt = min(ECHUNK, len(idxs) - c0)
                    if use8[k]:
                        j0 = f8_pos[idxs[c0]]
                        et = e8pool.tile([128, ECHUNK * KC], F8, tag="e8")
                        nc.sync.dma_start(
                            et[:, :cnt * KC],
                            emb8[:, j0 * KC:(j0 + cnt) * KC])
                    else:
                        j0 = f16_pos[idxs[c0]]
                        et = e16pool.tile([128, ECHUNK * KC], F16, tag="e16")
                        nc.sync.dma_start(
                            et[:, :cnt * KC],
                            emb16[:, j0 * KC:(j0 + cnt) * KC])
                    chunks.append((et, cnt))
                emb_tiles[k] = chunks

            def agg(k):
                g = groups[k]
                eng_name = assign[k]
                a = apool.tile([128, KC], F16, tag=f"acc_{k}")
                accs[k] = a
                chunks = emb_tiles.pop(k, [])
                if eng_name == "pe":
                    ps = psapool.tile([128, KC], F32, tag="psa")
                    # per-dt PSUM regions with uniform [128, BC] extents
                    nops = 1 + len(g["pairs"]) + len(g["emb"])
                    for dt in range(DT):
                        n = 1
                        nc.tensor.matmul(
                            ps[:, dt * BC:(dt + 1) * BC],
                            ident(g["self_coeff"]), ts(k, dt),
                            start=True, stop=(n == nops))
                        for src, coeff in g["pairs"]:
                            n += 1
                            nc.tensor.matmul(
                                ps[:, dt * BC:(dt + 1) * BC],
                                ident(coeff), ts(src, dt),
                                start=False, stop=(n == nops))
                        j = 0
                        for et, cnt in chunks:
                            for i in range(cnt):
                                n += 1
                                nc.tensor.matmul(
                                    ps[:, dt * BC:(dt + 1) * BC],
                                    id8s[:, :],
                                    et[:, i * KC + dt * BC:
                                       i * KC + (dt + 1) * BC],
                                    start=False, stop=(n == nops))
                                j += 1
                    nc.scalar.activation(a[:], ps[:], AF.Copy,
                                         bias=0.0, scale=1.0)
                    return
                eng = nc.vector if eng_name == "dve" else nc.gpsimd
                nc.vector.tensor_scalar_mul(a[:], ts(k), g["self_coeff"])
                for src, coeff in g["pairs"]:
                    if coeff == 1.0:
                        eng.tensor_add(a[:], a[:], ts(src))
                    elif coeff == -1.0:
                        eng.tensor_sub(a[:], a[:], ts(src))
                    elif eng_name == "pool" and float(coeff).is_integer():
                        op = (eng.tensor_add if coeff > 0
                              else eng.tensor_sub)
                        for _ in range(int(abs(coeff))):
                            op(a[:], a[:], ts(src))
                    else:
                        # TensorScalarPtr is DVE-only
                        nc.vector.scalar_tensor_tensor(
                            a[:], ts(src), coeff, a[:], OP.mult, OP.add)
                for et, cnt in chunks:
                    for i in range(cnt):
                        eng.tensor_add(a[:], a[:],
                                       et[:, i * KC:(i + 1) * KC])

            n_epi = plan["epi_dve"]
            dve_relu = set()
            if n_epi:
                step = max(1, (4 * (T // 2)) // n_epi)
                dve_relu = {(i // HT, i % HT)
                            for i in range(0, 4 * (T // 2), step)}

            def mlp(p):
                k0, k1 = 2 * p, 2 * p + 1
                hids = []
                for ht in range(HT):
                    ps = ps1pool.tile([128, 2 * BC], F32, tag="ps1")
                    for sub, k in ((0, k0), (1, k1)):
                        for dt in range(DT):
                            nc.tensor.matmul(
                                ps[:, sub * BC:(sub + 1) * BC],
                                w1s[:, dt * H + ht * 128:
                                    dt * H + (ht + 1) * 128],
                                accs[k][:, dt * BC:(dt + 1) * BC],
                                start=(dt == 0), stop=(dt == DT - 1))
                    hid = hpool.tile([128, 2 * BC], F16, tag="hid")
                    if (p, ht) in dve_relu:
                        nc.vector.tensor_scalar(
                            hid[:], ps[:], b1s[:, ht:ht + 1], 0.0,
                            OP.add, OP.max)
                    else:
                        nc.scalar.activation(hid[:], ps[:], AF.Relu,
                                             bias=b1s[:, ht:ht + 1], scale=1.0)
                    hids.append(hid)
                for dt2 in range(DT):
                    ps2 = ps2pool.tile([128, 2 * BC], F32, tag="ps2")
                    for sub in range(2):
                        for ht in range(HT):
                            nc.tensor.matmul(
                                ps2[:, sub * BC:(sub + 1) * BC],
                                w2s[:, ht * D + dt2 * 128:
                                    ht * D + (dt2 + 1) * 128],
                                hids[ht][:, sub * BC:(sub + 1) * BC],
                                start=(ht == 0), stop=(ht == HT - 1))
                    ot = opool.tile([128, 2 * BC], F16, tag="ot")
                    nc.scalar.activation(ot[:], ps2[:], AF.Identity,
                                         bias=b2s[:, dt2:dt2 + 1], scale=1.0)
                    nc.sync.dma_start(
                        outT[:, dt2 * T * BC + p * 2 * BC:
                             dt2 * T * BC + (p + 1) * 2 * BC], ot[:])

            # software pipeline: aggregate one pair ahead of the MLP so the
            # Act-engine merges / DVE adds for pair p+1 queue BEFORE pair p's
            # epilogue tiles and the PE stays fed.
            for k in range(4):
                load_emb(k)
            agg(0)
            agg(1)
            for p in range(T // 2):
                if 2 * p + 4 < T:
                    load_emb(2 * p + 4)
                if 2 * p + 5 < T:
                    load_emb(2 * p + 5)
                if 2 * p + 2 < T:
                    agg(2 * p + 2)
                    agg(2 * p + 3)
                mlp(p)

        if loop:
            with tc.For_i(0, loop, 1,
                          hint_engines=(ET.PE, ET.DVE, ET.Activation, ET.SP)):
                for rep in range(repeats):
                    body(rep)
        else:
            for rep in range(repeats):
                body(rep)

    nc.compile()
    _KERNEL_CACHE[key] = nc
    return nc


def _prep_inputs(term_emb, pred_emb, inv_pred_emb, W1, b1, W2, b2, msgs):
    """Shard/transpose/cast host-side into per-core device layouts."""
    import ml_dtypes
    plan = _plan(tuple(msgs))
    f16_order, f8_order = plan["f16_order"], plan["f8_order"]
    scales = plan["scales"]

    def fuse(a):
        # [X, BC, D] -> [128, X*KC] with column = x*KC + dt*BC + b
        X = a.shape[0]
        a2 = a.transpose(0, 2, 1).reshape(X, DT, 128, BC)
        return np.ascontiguousarray(
            a2.transpose(2, 0, 1, 3).reshape(128, X * KC))

    w1t = np.ascontiguousarray(
        W1.astype(np.float16).reshape(DT, 128, H).transpose(1, 0, 2)
        .reshape(128, DT * H))
    w2t = np.ascontiguousarray(
        W2.astype(np.float16).reshape(HT, 128, D).transpose(1, 0, 2)
        .reshape(128, HT * D))
    b1t = np.ascontiguousarray(b1.astype(np.float32).reshape(HT, 128).T)
    b2t = np.ascontiguousarray(b2.astype(np.float32).reshape(DT, 128).T)
    identT = np.zeros((128, max(len(scales), 1) * 128), np.float16)
    for j, c in enumerate(scales):
        identT[:, j * 128:(j + 1) * 128] = np.eye(128, dtype=np.float16) * c
    ident8 = np.ascontiguousarray(np.eye(128).astype(ml_dtypes.float8_e3m4))

    msg_arrs = []
    for (dst, src, s, which, e) in plan["msgs"]:
        arr = pred_emb if which == 0 else inv_pred_emb
        msg_arrs.append((arr, e, s))

    in_maps = []
    for c in range(N_CORES):
        sl = slice(c * BC, (c + 1) * BC)
        termTc = fuse(term_emb[:, sl, :].astype(np.float16))
        n16 = max(len(f16_order), 1)
        n8 = max(len(f8_order), 1)
        e16 = np.zeros((128, n16 * KC), np.float16)
        e8 = np.zeros((128, n8 * KC), ml_dtypes.float8_e3m4)
        for j, mi in enumerate(f16_order):
            arr, e, s = msg_arrs[mi]
            a = arr[e, sl, :][None].astype(np.float32)
            if s != 1.0:
                a = a * s
            e16[:, j * KC:(j + 1) * KC] = fuse(a.astype(np.float16))
        for j, mi in enumerate(f8_order):
            arr, e, s = msg_arrs[mi]
            a = arr[e, sl, :][None].astype(np.float32)
            if s != 1.0:
                a = a * s
            e8[:, j * KC:(j + 1) * KC] = fuse(a).astype(ml_dtypes.float8_e3m4)
        in_maps.append(dict(termT=termTc, emb16=e16, emb8=e8, w1t=w1t,
                            w2t=w2t, b1t=b1t, b2t=b2t, identT=identT,
                            ident8=ident8))
    return in_maps


def kernel(term_emb, pred_emb, inv_pred_emb, signs, W1, b1, W2, b2,
           heads, tails):
    term_emb = np.asarray(term_emb, dtype=np.float32)
    pred_emb = np.asarray(pred_emb, dtype=np.float32)
    inv_pred_emb = np.asarray(inv_pred_emb, dtype=np.float32)
    signs = np.asarray(signs, dtype=np.float32)
    W1 = np.asarray(W1, dtype=np.float32)
    b1 = np.asarray(b1, dtype=np.float32)
    W2 = np.asarray(W2, dtype=np.float32)
    b2 = np.asarray(b2, dtype=np.float32)
    heads = np.asarray(heads).astype(np.int64)
    tails = np.asarray(tails).astype(np.int64)

    msgs = _messages(heads, tails, signs)
    nc = _build(tuple(msgs))
    in_maps = _prep_inputs(term_emb, pred_emb, inv_pred_emb, W1, b1, W2, b2,
                           msgs)
    res = run_bass_kernel_spmd(nc, in_maps, list(range(N_CORES)))

    out = np.empty((T, B, D), np.float32)
    for c in range(N_CORES):
        o = res.results[c]["outT"].astype(np.float32)
        # o[p, dt*T*BC + k*BC + b] -> out[k, c*BC+b, dt*128+p]
        o4 = o.reshape(128, DT, T, BC).transpose(2, 3, 1, 0)  # [T, BC, DT, 128]
        out[:, c * BC:(c + 1) * BC, :] = o4.reshape(T, BC, D)
    return out


# revision 21
# speedup vs baseline: 1.1727x; 1.0169x over previous
"""Trainium2 Bass kernel for nn_LogicalGNNLayer (GNN message passing + MLP).

Computation (reference):
    h = term_emb[heads]; t = term_emb[tails]           # gather  [E,B,D]
    agg = segsum(s*(h+pred), tails) + segsum(s*(t+inv), heads)   # [T,B,D]
    agg += EPS*term_emb
    out = relu(agg @ W1 + b1) @ W2 + b2                # [T,B,D]

Strategy (v1 rewrite):
  - Batch B sharded across 8 cores (Bc=512); term/edge structure and weights
    replicated.  heads/tails/signs are read on the host and the message
    structure is baked into the program.
  - Fused on-chip layout: 128 partitions hold d-within-tile, the free axis
    holds (k, dt, b) for terms (so each term/acc is one contiguous
    [128, 1024] span) and (dt, k, b) for the output.
  - The aggregation acc[k] = (EPS + selfs)*term[k] + sum c*term[src] + sum emb
    is elementwise work balanced across THREE engines (HW-calibrated costs):
      * DVE:  tensor ops at 2x fp16 mode (~716ns per [128,1024] add)
      * Pool: tensor ops (~2.08us/add; slow but otherwise idle)
      * PE:   scaled-identity matmuls accumulating in PSUM (~174ns/512 cols),
              merged to SBUF by one Act-engine copy per group
  - emb tensors for PE/Pool groups are cast to fp8e3 (e3m4) on the host,
    halving their DMA traffic at no engine cost; DVE groups keep fp16 (fp8
    operands drop DVE to 1x mode).  Per-group dtype is a balance knob.
  - MLP: fp16 matmuls with fp32 PSUM accumulation; ReLU+b1 / +b2 epilogues
    on the Act engine via bias APs (nonzero biases handled), with an
    adjustable DVE share.
  - Output stored fp16 (cast to fp32 on the host).
"""

import numpy as np

import concourse.bass as bass
import concourse.tile as tile
from concourse import bacc, mybir
from concourse.bass_utils import run_bass_kernel_spmd

T, B, D, H, E = 16, 4096, 256, 512, 32
EPS = 0.1
N_CORES = 8
BC = B // N_CORES            # 512 batch per core
DT = D // 128                # 2 d-tiles
HT = H // 128                # 4 h-tiles
KC = DT * BC                 # 1024 columns per term (dt-fused)
NCOL = T * KC                # 16384 free-axis columns
ECHUNK = 8                   # messages per emb DMA/tile
F16 = mybir.dt.float16
F32 = mybir.dt.float32
F8 = mybir.dt.float8e3       # e3m4: 4 mantissa bits
AF = mybir.ActivationFunctionType
OP = mybir.AluOpType
ET = mybir.EngineType

# HW-calibrated per-op costs in us ([128,1024]-equivalent; microbenched on
# the axon-tunneled trn2):
C_DVE_ADD = 0.716
C_DVE_ADD8 = 1.295
C_DVE_STT = 1.35
C_DVE_INIT = 0.39
C_DVE_EPI = 1.43
C_POOL_ADD = 2.08
C_POOL_STT = 1.55
C_PE_512 = 0.174
C_ACT_TILE = 1.04
C_MLP = 44.5
C_ACT_FIXED = 48 * 1.037     # 32 relu + 16 out tiles, all on Act
US_PER_MIB = 2.697
DMA_FIXED_MIB = 4.0 + 4.0 + 0.625 + 0.07   # term + out + weights + ident
C_SP_DMA = 0.565

_KERNEL_CACHE = {}


def _messages(heads, tails, signs):
    """Directed message list (dst, src, sign, which_emb, e)."""
    msgs = []
    for e in range(E):
        h, t, s = int(heads[e]), int(tails[e]), float(signs[e])
        assert 0 <= h < T and 0 <= t < T
        msgs.append((t, h, s, 0, e))   # msg_to_tail: acc[t] += s*(term[h]+pred[e])
        msgs.append((h, t, s, 1, e))   # msg_to_head: acc[h] += s*(term[t]+inv[e])
    return msgs


def _plan(msgs_key):
    """Group messages by destination, assign each group to an engine, lay
    out the emb DRAM ordering, and pick the epilogue split — all from
    HW-calibrated cost estimates."""
    msgs = list(msgs_key)
    groups = []
    for k in range(T):
        mlist = [(i, m) for i, m in enumerate(msgs) if m[0] == k]
        self_coeff = EPS + sum(m[2] for _, m in mlist if m[1] == k)
        pair_coeffs = {}
        for _, (_d, src, s, _w, _e) in mlist:
            if src != k:
                pair_coeffs[src] = pair_coeffs.get(src, 0.0) + s
        pairs = [(src, c) for src, c in sorted(pair_coeffs.items()) if c != 0.0]
        emb = [i for i, _m in mlist]   # every message carries its emb term
        groups.append(dict(k=k, self_coeff=self_coeff, pairs=pairs, emb=emb))

    def dve_cost(g):
        c = sum(C_DVE_ADD if abs(co) == 1.0 else C_DVE_STT
                for _s, co in g["pairs"])
        return c + len(g["emb"]) * C_DVE_ADD

    def pool_cost(g):
        # Pool lacks TensorScalarPtr: |coeff|==c emitted as c adds/subs
        c = sum(abs(co) * C_POOL_ADD if float(co).is_integer() else C_DVE_STT
                for _s, co in g["pairs"])
        return c + len(g["emb"]) * C_POOL_ADD

    def pe_cost(g):
        return (2 + 2 * len(g["pairs"]) + 2 * len(g["emb"])) * C_PE_512

    est = {"pe": C_MLP, "dve": T * C_DVE_INIT, "pool": 0.0,
           "act": C_ACT_FIXED, "dma": 0.0, "sp": 40 * C_SP_DMA}
    emb_mib = 0.0
    assign = {}
    order = sorted(range(T), key=lambda k: -dve_cost(groups[k]))
    for k in order:
        g = groups[k]
        nmsg = len(g["emb"])
        dma16 = (DMA_FIXED_MIB + emb_mib + 0.25 * nmsg) * US_PER_MIB
        dma8 = (DMA_FIXED_MIB + emb_mib + 0.125 * nmsg) * US_PER_MIB
        cand = {
            "dve": max(est["pe"], est["dve"] + dve_cost(g), est["pool"],
                       est["act"], dma16),
            # Pool reading fp8 crashes the device (NRT_EXEC_UNIT_UNRECOVERABLE)
            # so pool groups keep fp16 emb.
            "pool": max(est["pe"], est["dve"], est["pool"] + pool_cost(g),
                        est["act"], dma16),
            "pe": max(est["pe"] + pe_cost(g), est["dve"] - C_DVE_INIT,
                      est["pool"], est["act"] + C_ACT_TILE, dma8),
        }
        eng = min(cand, key=lambda e: cand[e])
        assign[k] = eng
        if eng == "dve":
            est["dve"] += dve_cost(g)
            emb_mib += 0.25 * nmsg
        elif eng == "pool":
            est["pool"] += pool_cost(g)
            emb_mib += 0.25 * nmsg
        else:
            est["pe"] += pe_cost(g)
            est["act"] += C_ACT_TILE
            est["dve"] -= C_DVE_INIT
            emb_mib += 0.125 * nmsg
    est["dma"] = (DMA_FIXED_MIB + emb_mib) * US_PER_MIB

    # HW: gappy PE matmul streams ramp-throttle (p-state), so the PE runs
    # ~30% over the warm estimate; shed the smallest PE groups to the
    # underloaded Pool engine (their emb flips fp8 -> fp16).
    pe_shed = 2
    for _ in range(pe_shed):
        pe_ks = [k for k in range(T) if assign[k] == "pe"]
        if len(pe_ks) <= 1:
            break
        k = min(pe_ks, key=lambda kk: pe_cost(groups[kk]))
        g = groups[k]
        if est["pool"] + pool_cost(g) > 52.0:
            break
        assign[k] = "pool"
        est["pe"] -= pe_cost(g)
        est["pool"] += pool_cost(g)
        est["act"] -= C_ACT_TILE
        est["dve"] += C_DVE_INIT
        emb_mib += 0.125 * len(g["emb"])
        est["dma"] = (DMA_FIXED_MIB + emb_mib) * US_PER_MIB

    # epilogue split: move tiles Act -> DVE while the makespan improves
    epi_dve = 0
    while epi_dve < 24:
        cur = max(est.values())
        nxt_act, nxt_dve = est["act"] - 1.037, est["dve"] + C_DVE_EPI
        if max(est["pe"], est["pool"], est["dma"], est["sp"],
               nxt_act, nxt_dve) < cur:
            est["act"], est["dve"] = nxt_act, nxt_dve
            epi_dve += 1
        else:
            break

    # flip DVE-group emb to fp8 while DMA dominates (NOT pool: fp8 on the
    # Pool engine crashes the device)
    use8 = {k: assign[k] == "pe" for k in range(T)}
    for k in order:
        if assign[k] != "dve":
            continue
        nmsg = len(groups[k]["emb"])
        if nmsg and est["dma"] == max(est.values()):
            nxt_dve = est["dve"] + (C_DVE_ADD8 - C_DVE_ADD) * nmsg
            nxt_dma = est["dma"] - 0.125 * nmsg * US_PER_MIB
            if max(nxt_dve, nxt_dma) < max(est["dve"], est["dma"]):
                est["dve"], est["dma"] = nxt_dve, nxt_dma
                use8[k] = True

    # emb DRAM layouts: grouped by k in processing order
    f16_order, f8_order = [], []
    for k in range(T):
        (f8_order if use8[k] else f16_order).extend(groups[k]["emb"])

    scales = []
    for k in range(T):
        if assign[k] != "pe":
            continue
        g = groups[k]
        for c in [g["self_coeff"]] + [c for _s, c in g["pairs"]]:
            if c not in scales:
                scales.append(c)

    return dict(groups=groups, assign=assign, use8=use8,
                f16_order=f16_order, f8_order=f8_order, scales=scales,
                epi_dve=epi_dve, est=est, msgs=msgs)


def _build(msgs_key, repeats=1, loop=0):
    key = (msgs_key, repeats, loop)
    if key in _KERNEL_CACHE:
        return _KERNEL_CACHE[key]
    plan = _plan(msgs_key)
    groups, assign, use8 = plan["groups"], plan["assign"], plan["use8"]
    f16_pos = {m: j for j, m in enumerate(plan["f16_order"])}
    f8_pos = {m: j for j, m in enumerate(plan["f8_order"])}
    scales = plan["scales"]
    n16, n8 = len(plan["f16_order"]), len(plan["f8_order"])

    nc = bacc.Bacc("TRN2", target_bir_lowering=False, debug=False,
                   num_devices=N_CORES)
    termT = nc.declare_dram_parameter("termT", [128, NCOL], F16, isOutput=False)
    emb16 = nc.declare_dram_parameter("emb16", [128, max(n16, 1) * KC], F16,
                                      isOutput=False)
    emb8 = nc.declare_dram_parameter("emb8", [128, max(n8, 1) * KC], F8,
                                     isOutput=False)
    w1d = nc.declare_dram_parameter("w1t", [128, DT * H], F16, isOutput=False)
    w2d = nc.declare_dram_parameter("w2t", [128, HT * D], F16, isOutput=False)
    b1d = nc.declare_dram_parameter("b1t", [128, HT], F32, isOutput=False)
    b2d = nc.declare_dram_parameter("b2t", [128, DT], F32, isOutput=False)
    idd = nc.declare_dram_parameter("identT", [128, max(len(scales), 1) * 128],
                                    F16, isOutput=False)
    id8d = nc.declare_dram_parameter("ident8", [128, 128], F8, isOutput=False)
    outT = nc.declare_dram_parameter("outT", [128, NCOL], F16, isOutput=True)

    with nc.allow_low_precision(reason="fp16/fp8 on-chip"), \
            tile.TileContext(nc) as tc, \
            tc.tile_pool(name="const", bufs=1) as cpool, \
            tc.tile_pool(name="term", bufs=2) as tpool, \
            tc.tile_pool(name="acc", bufs=1) as apool, \
            tc.tile_pool(name="e16p", bufs=3) as e16pool, \
            tc.tile_pool(name="e8p", bufs=2) as e8pool, \
            tc.tile_pool(name="hid", bufs=8) as hpool, \
            tc.tile_pool(name="out", bufs=4) as opool, \
            tc.tile_pool(name="ps1", bufs=2, space="PSUM") as ps1pool, \
            tc.tile_pool(name="ps2", bufs=1, space="PSUM") as ps2pool, \
            tc.tile_pool(name="psa", bufs=1, space="PSUM") as psapool:

        # ---- persistent loads -------------------------------------------
        w1s = cpool.tile([128, DT * H], F16, tag="w1")
        nc.sync.dma_start(w1s[:], w1d[:, :])
        w2s = cpool.tile([128, HT * D], F16, tag="w2")
        nc.sync.dma_start(w2s[:], w2d[:, :])
        b1s = cpool.tile([128, HT], F32, tag="b1")
        nc.sync.dma_start(b1s[:], b1d[:, :])
        b2s = cpool.tile([128, DT], F32, tag="b2")
        nc.sync.dma_start(b2s[:], b2d[:, :])
        ids = None
        if scales:
            ids = cpool.tile([128, len(scales) * 128], F16, tag="ident")
            nc.sync.dma_start(ids[:], idd[:, :])
        id8s = None
        if n8:
            id8s = cpool.tile([128, 128], F8, tag="ident8")
            nc.sync.dma_start(id8s[:], id8d[:, :])

        def ident(coeff):
            j = scales.index(coeff)
            return ids[:, j * 128:(j + 1) * 128]

        def body(rep):
            tt = tpool.tile([128, NCOL], F16, tag="term")
            nc.sync.dma_start(tt[:], termT[:, :])

            def ts(k, dt=None):
                if dt is None:
                    return tt[:, k * KC:(k + 1) * KC]
                return tt[:, k * KC + dt * BC:k * KC + (dt + 1) * BC]

            accs = [None] * T
            emb_tiles = {}

            def load_emb(k):
                g = groups[k]
                chunks = []
                idxs = g["emb"]
                for c0 in range(0, len(idxs), ECHUNK):
                    cnt = min(ECHUNK, len(idxs) - c0)
                    if use8[k]:
                        j0 = f8_pos[idxs[c0]]
                        et = e8pool.tile([128, ECHUNK * KC], F8, tag="e8")
                        nc.sync.dma_start(
                            et[:, :cnt * KC],
                            emb8[:, j0 * KC:(j0 + cnt) * KC])
                    else:
                        j0 = f16_pos[idxs[c0]]
                        et = e16pool.tile([128, ECHUNK * KC], F16, tag="e16")
                        nc.sync.dma_start(
                            et[:, :cnt * KC],
                            emb16[:, j0 * KC:(j0 + cnt) * KC])
                    chunks.append((et, cnt))
                emb_tiles[k] = chunks

            def agg(k):
                g = groups[k]
                eng_name = assign[k]
                a = apool.tile([128, KC], F16, tag=f"acc_{k}")
                accs[k] = a
                chunks = emb_tiles.pop(k, [])
                if eng_name == "pe":
                    ps = psapool.tile([128, KC], F32, tag="psa")
                    # per-dt PSUM regions with uniform [128, BC] extents
                    nops = 1 + len(g["pairs"]) + len(g["emb"])
                    for dt in range(DT):
                        n = 1
                        nc.tensor.matmul(
                            ps[:, dt * BC:(dt + 1) * BC],
                            ident(g["self_coeff"]), ts(k, dt),
                            start=True, stop=(n == nops))
                        for src, coeff in g["pairs"]:
                            n += 1
                            nc.tensor.matmul(
                                ps[:, dt * BC:(dt + 1) * BC],
                                ident(coeff), ts(src, dt),
                                start=False, stop=(n == nops))
                        j = 0
                        for et, cnt in chunks:
                            for i in range(cnt):
                                n += 1
                                nc.tensor.matmul(
                                    ps[:, dt * BC:(dt + 1) * BC],
                                    id8s[:, :],
                                    et[:, i * KC + dt * BC:
                                       i * KC + (dt + 1) * BC],
                                    start=False, stop=(n == nops))
                                j += 1
                    nc.scalar.activation(a[:], ps[:], AF.Copy,
                                         bias=0.0, scale=1.0)
                    return
                eng = nc.vector if eng_name == "dve" else nc.gpsimd
                nc.vector.tensor_scalar_mul(a[:], ts(k), g["self_coeff"])
                for src, coeff in g["pairs"]:
                    if coeff == 1.0:
                        eng.tensor_add(a[:], a[:], ts(src))
                    elif coeff == -1.0:
                        eng.tensor_sub(a[:], a[:], ts(src))
                    elif eng_name == "pool" and float(coeff).is_integer():
                        op = (eng.tensor_add if coeff > 0
                              else eng.tensor_sub)
                        for _ in range(int(abs(coeff))):
                            op(a[:], a[:], ts(src))
                    else:
                        # TensorScalarPtr is DVE-only
                        nc.vector.scalar_tensor_tensor(
                            a[:], ts(src), coeff, a[:], OP.mult, OP.add)
                for et, cnt in chunks:
                    for i in range(cnt):
                        eng.tensor_add(a[:], a[:],
                                       et[:, i * KC:(i + 1) * KC])

            n_epi = plan["epi_dve"]
            dve_relu = set()
            if n_epi:
                step = max(1, (4 * (T // 2)) // n_epi)
                dve_relu = {(i // HT, i % HT)
                            for i in range(0, 4 * (T // 2), step)}

            def mlp(p):
                k0, k1 = 2 * p, 2 * p + 1
                hids = []
                for ht in range(HT):
                    ps = ps1pool.tile([128, 2 * BC], F32, tag="ps1")
                    for sub, k in ((0, k0), (1, k1)):
                        for dt in range(DT):
                            nc.tensor.matmul(
                                ps[:, sub * BC:(sub + 1) * BC],
                                w1s[:, dt * H + ht * 128:
                                    dt * H + (ht + 1) * 128],
                                accs[k][:, dt * BC:(dt + 1) * BC],
                                start=(dt == 0), stop=(dt == DT - 1))
                    hid = hpool.tile([128, 2 * BC], F16, tag="hid")
                    if (p, ht) in dve_relu:
                        nc.vector.tensor_scalar(
                            hid[:], ps[:], b1s[:, ht:ht + 1], 0.0,
                            OP.add, OP.max)
                    else:
                        nc.scalar.activation(hid[:], ps[:], AF.Relu,
                                             bias=b1s[:, ht:ht + 1], scale=1.0)
                    hids.append(hid)
                for dt2 in range(DT):
                    ps2 = ps2pool.tile([128, 2 * BC], F32, tag="ps2")
                    for sub in range(2):
                        for ht in range(HT):
                            nc.tensor.matmul(
                                ps2[:, sub * BC:(sub + 1) * BC],
                                w2s[:, ht * D + dt2 * 128:
                                    ht * D + (dt2 + 1) * 128],
                                hids[ht][:, sub * BC:(sub + 1) * BC],
                                start=(ht == 0), stop=(ht == HT - 1))
                    ot = opool.tile([128, 2 * BC], F16, tag="ot")
                    nc.scalar.activation(ot[:], ps2[:], AF.Identity,
                                         bias=b2s[:, dt2:dt2 + 1], scale=1.0)
                    nc.sync.dma_start(
                        outT[:, dt2 * T * BC + p * 2 * BC:
                             dt2 * T * BC + (p + 1) * 2 * BC], ot[:])

            # software pipeline: aggregate one pair ahead of the MLP so the
            # Act-engine merges / DVE adds for pair p+1 queue BEFORE pair p's
            # epilogue tiles and the PE stays fed.
            for k in range(4):
                load_emb(k)
            agg(0)
            agg(1)
            for p in range(T // 2):
                if 2 * p + 4 < T:
                    load_emb(2 * p + 4)
                if 2 * p + 5 < T:
                    load_emb(2 * p + 5)
                if 2 * p + 2 < T:
                    agg(2 * p + 2)
                    agg(2 * p + 3)
                mlp(p)

        if loop:
            with tc.For_i(0, loop, 1,
                          hint_engines=(ET.PE, ET.DVE, ET.Activation, ET.SP)):
                for rep in range(repeats):
                    body(rep)
        else:
            for rep in range(repeats):
                body(rep)

    nc.compile()
    _KERNEL_CACHE[key] = nc
    return nc


def _prep_inputs(term_emb, pred_emb, inv_pred_emb, W1, b1, W2, b2, msgs):
    """Shard/transpose/cast host-side into per-core device layouts."""
    import ml_dtypes
    plan = _plan(tuple(msgs))
    f16_order, f8_order = plan["f16_order"], plan["f8_order"]
    scales = plan["scales"]

    def fuse(a):
        # [X, BC, D] -> [128, X*KC] with column = x*KC + dt*BC + b
        X = a.shape[0]
        a2 = a.transpose(0, 2, 1).reshape(X, DT, 128, BC)
        return np.ascontiguousarray(
            a2.transpose(2, 0, 1, 3).reshape(128, X * KC))

    w1t = np.ascontiguousarray(
        W1.astype(np.float16).reshape(DT, 128, H).transpose(1, 0, 2)
        .reshape(128, DT * H))
    w2t = np.ascontiguousarray(
        W2.astype(np.float16).reshape(HT, 128, D).transpose(1, 0, 2)
        .reshape(128, HT * D))
    b1t = np.ascontiguousarray(b1.astype(np.float32).reshape(HT, 128).T)
    b2t = np.ascontiguousarray(b2.astype(np.float32).reshape(DT, 128).T)
    identT = np.zeros((128, max(len(scales), 1) * 128), np.float16)
    for j, c in enumerate(scales):
        identT[:, j * 128:(j + 1) * 128] = np.eye(128, dtype=np.float16) * c
    ident8 = np.ascontiguousarray(np.eye(128).astype(ml_dtypes.float8_e3m4))

    msg_arrs = []
    for (dst, src, s, which, e) in plan["msgs"]:
        arr = pred_emb if which == 0 else inv_pred_emb
        msg_arrs.append((arr, e, s))

    in_maps = []
    for c in range(N_CORES):
        sl = slice(c * BC, (c + 1) * BC)
        termTc = fuse(term_emb[:, sl, :].astype(np.float16))
        n16 = max(len(f16_order), 1)
        n8 = max(len(f8_order), 1)
        e16 = np.zeros((128, n16 * KC), np.float16)
        e8 = np.zeros((128, n8 * KC), ml_dtypes.float8_e3m4)
        for j, mi in enumerate(f16_order):
            arr, e, s = msg_arrs[mi]
            a = arr[e, sl, :][None].astype(np.float32)
            if s != 1.0:
                a = a * s
            e16[:, j * KC:(j + 1) * KC] = fuse(a.astype(np.float16))
        for j, mi in enumerate(f8_order):
            arr, e, s = msg_arrs[mi]
            a = arr[e, sl, :][None].astype(np.float32)
            if s != 1.0:
                a = a * s
            e8[:, j * KC:(j + 1) * KC] = fuse(a).astype(ml_dtypes.float8_e3m4)
        in_maps.append(dict(termT=termTc, emb16=e16, emb8=e8, w1t=w1t,
                            w2t=w2t, b1t=b1t, b2t=b2t, identT=identT,
                            ident8=ident8))
    return in_maps


def kernel(term_emb, pred_emb, inv_pred_emb, signs, W1, b1, W2, b2,
           heads, tails):
    term_emb = np.asarray(term_emb, dtype=np.float32)
    pred_emb = np.asarray(pred_emb, dtype=np.float32)
    inv_pred_emb = np.asarray(inv_pred_emb, dtype=np.float32)
    signs = np.asarray(signs, dtype=np.float32)
    W1 = np.asarray(W1, dtype=np.float32)
    b1 = np.asarray(b1, dtype=np.float32)
    W2 = np.asarray(W2, dtype=np.float32)
    b2 = np.asarray(b2, dtype=np.float32)
    heads = np.asarray(heads).astype(np.int64)
    tails = np.asarray(tails).astype(np.int64)

    msgs = _messages(heads, tails, signs)
    nc = _build(tuple(msgs))
    in_maps = _prep_inputs(term_emb, pred_emb, inv_pred_emb, W1, b1, W2, b2,
                           msgs)
    res = run_bass_kernel_spmd(nc, in_maps, list(range(N_CORES)))

    out = np.empty((T, B, D), np.float32)
    for c in range(N_CORES):
        o = res.results[c]["outT"].astype(np.float32)
        # o[p, dt*T*BC + k*BC + b] -> out[k, c*BC+b, dt*128+p]
        o4 = o.reshape(128, DT, T, BC).transpose(2, 3, 1, 0)  # [T, BC, DT, 128]
        out[:, c * BC:(c + 1) * BC, :] = o4.reshape(T, BC, D)
    return out
